# revision 1
# baseline (speedup 1.0000x reference)
"""Trainium2 Bass kernel for nn_MultiHeadAttention (B=2, L=2048, H=768, 12 heads).

Sharding (8 cores): core c -> batch b=c//4, heads 3*(c%4)..3*(c%4)+2.
Each core: QKV proj for its 3 heads, flash-style attention (scores^T layout,
key-mask folded into V', query-mask folded into 1/l), partial output
projection with wo rows (row-parallel) + x/4 residual, ReduceScatter(add)
over the 4 cores of its batch, then layernorm over the sequence dim on its
192-row hidden slice. Host assembles [2,2048,768] from 8 [192,2048] slices.

PSUM static budget (8 banks): tag s = 2 bufs x [128,1024] (4 banks, shared by
scores / transposes / projections), tag av = [65,1024] (2), tag rb = [64,1024]
(2).
"""

import sys

import ml_dtypes
import numpy as np

BFNP = ml_dtypes.bfloat16

sys.path.insert(0, "/opt/trn_rl_repo")

import concourse.bass as bass  # noqa: E402
import concourse.bacc as bacc  # noqa: E402
import concourse.mybir as mybir  # noqa: E402
from concourse import tile  # noqa: E402
from concourse.bass_utils import run_bass_kernel_spmd  # noqa: E402

F32 = mybir.dt.float32
BF16 = mybir.dt.bfloat16
I32 = mybir.dt.int32
AF = mybir.ActivationFunctionType
ALU = mybir.AluOpType

HIDDEN = 768
HEADS = 12
HD = 64
L = 2048
B = 2
NCORES = 8
HPC = 3          # heads per core
HF = HPC * HD    # 192 features per core
LT = L // 128    # 16 l-tiles
HC = HIDDEN // 128  # 6 hidden chunks
OSL = HIDDEN // 4   # 192 output-slice rows per core


def build_nc():
    nc = bacc.Bacc("TRN2", target_bir_lowering=False, debug=False,
                   num_devices=NCORES)

    x_d = nc.dram_tensor("x", [L, HIDDEN], F32, kind="ExternalInput")
    wq_d = nc.dram_tensor("wq", [HIDDEN, HF], BF16, kind="ExternalInput")
    wk_d = nc.dram_tensor("wk", [HIDDEN, HF], BF16, kind="ExternalInput")
    wv_d = nc.dram_tensor("wv", [HIDDEN, HF], BF16, kind="ExternalInput")
    wo_d = nc.dram_tensor("wo_r", [HF, HIDDEN], BF16, kind="ExternalInput")
    mask_d = nc.dram_tensor("mask_i", [1, L], I32, kind="ExternalInput")
    # params_col[128, 16]: cols 0,1=wq_b(192) 2,3=wk_b 4,5=wv_b 6..11=wo_b/4
    # (768), 12,13=gamma slice, 14,15=beta slice
    pcol_d = nc.dram_tensor("params_col", [128, 16], F32, kind="ExternalInput")
    # params_row[1, 960]: 0:192 wv_b, 192:960 wo_b/4
    prow_d = nc.dram_tensor("params_row", [1, 960], BF16, kind="ExternalInput")
    xr_d = nc.dram_tensor("xr", [L, OSL], F32, kind="ExternalInput")
    out_d = nc.dram_tensor("out_t", [OSL, L], F32, kind="ExternalOutput")

    partial_d = nc.dram_tensor("partial_acc", [HIDDEN, L], F32)
    rs_d = nc.dram_tensor("rs_out", [OSL * L], F32)

    with tile.TileContext(nc) as tc:
        with (
            tc.tile_pool(name="persist", bufs=1) as pers,
            tc.tile_pool(name="xin", bufs=3) as xin,
            tc.tile_pool(name="work", bufs=2) as work,
            tc.tile_pool(name="ps2", bufs=2, space=bass.MemorySpace.PSUM) as ps2,
            tc.tile_pool(name="pav", bufs=2, space=bass.MemorySpace.PSUM) as pav,
            tc.tile_pool(name="pexp", bufs=3) as pexp,
        ):
            def ps_tile(shape, name):
                return ps2.tile(shape, F32, tag="s", name=name,
                                padded_shape=[128, 1024])

            # ---------- phase 0: constants ----------
            ident_i = pers.tile([128, 128], I32, tag="ident_i")
            nc.gpsimd.iota(ident_i[:], pattern=[[-1, 128]], base=0,
                           channel_multiplier=1)
            ident = pers.tile([128, 128], F32, tag="ident")
            nc.vector.tensor_scalar(
                ident[:], ident_i[:], 0, None, op0=ALU.is_equal
            )
            ones_row = pers.tile([1, 512], F32, tag="ones_row")
            nc.vector.memset(ones_row[:], 1.0)
            ones_bf = pers.tile([1, 512], BF16, tag="ones_bf")
            nc.vector.memset(ones_bf[:], 1.0)

            pcol = pers.tile([128, 16], F32, tag="pcol")
            nc.sync.dma_start(out=pcol[:], in_=pcol_d[:])
            prow = pers.tile([1, 960], BF16, tag="prow")
            nc.sync.dma_start(out=prow[:], in_=prow_d[:])

            mask_i = xin.tile([1, L], I32, tag="mask_i", bufs=1)
            nc.sync.dma_start(out=mask_i[:], in_=mask_d[:])
            mask_row = pers.tile([1, L], F32, tag="mask_row")
            nc.vector.tensor_copy(mask_row[:], mask_i[:])

            # mask columns [128, 16]: col t = mask[128t:128t+128]
            mask_cols = pers.tile([128, LT], F32, tag="mask_cols")
            for t in range(LT):
                mp = ps_tile([128, 1], f"mask_ps{t}")
                nc.tensor.matmul(
                    mp[:], mask_row[:, 128 * t:128 * (t + 1)], ones_row[:, 0:1]
                )
                nc.vector.tensor_copy(mask_cols[:, t:t + 1], mp[:])

            # query-mask broadcast over 64 partitions, built once
            mask_bc = pers.tile([64, L], BF16, tag="mask_bc")
            for i in range(2):
                mb = ps_tile([64, 1024], f"mb{i}")
                for j in range(2):
                    nc.tensor.matmul(
                        mb[:, 512 * j:512 * (j + 1)],
                        ones_row[:, 0:64],
                        mask_row[:, 1024 * i + 512 * j:1024 * i + 512 * (j + 1)],
                    )
                nc.vector.tensor_copy(mask_bc[:, 1024 * i:1024 * (i + 1)], mb[:])

            # weights loaded early; tiny PE "touch" matmuls absorb each DMA
            # lane wait so later matmuls stay under the 2-wait limit
            wq = pers.tile([128, HC, HF], BF16, tag="wq")
            wk = pers.tile([128, HC, HF], BF16, tag="wk")
            wv = pers.tile([128, HC, HF], BF16, tag="wv")
            for w_sb, w_d in ((wq, wq_d), (wk, wk_d), (wv, wv_d)):
                nc.sync.dma_start(
                    out=w_sb[:], in_=w_d[:].rearrange("(c p) m -> p c m", p=128)
                )
            wo_a = pers.tile([128, HIDDEN], BF16, tag="wo_a")
            wo_b_sb = pers.tile([64, HIDDEN], BF16, tag="wo_b")
            nc.sync.dma_start(out=wo_a[:], in_=wo_d[0:128, :])
            nc.sync.dma_start(out=wo_b_sb[:], in_=wo_d[128:192, :])
            touch_srcs = (wq[:, 0, 0:1], wk[:, 0, 0:1], wv[:, 0, 0:1],
                          wo_a[:, 0:1], wo_b_sb[:, 0:1], prow[:, 0:1])
            tch = pav.tile([1, 1], F32, tag="av", name="touch",
                           padded_shape=[65, 1024])
            for ti, tsr in enumerate(touch_srcs):
                nc.tensor.matmul(tch[:], tsr, tsr, start=(ti == 0),
                                 stop=(ti == len(touch_srcs) - 1),
                                 skip_group_check=True)
            tch_scr = work.tile([1, 1], F32, tag="tch_scr", bufs=1)
            nc.scalar.copy(tch_scr[:], tch[:])

            # ---------- phase 1: load x, build x^T ----------
            x_t = [pers.tile([128, L], BF16, tag=f"x_t{c}", name=f"x_t{c}")
                   for c in range(HC)]
            ident_b = pers.tile([128, 128], BF16, tag="ident_b")
            nc.vector.tensor_copy(ident_b[:], ident[:])
            for lt in range(LT):
                xn = xin.tile([128, HIDDEN], F32, tag="x_nat")
                nc.gpsimd.dma_start(out=xn[:], in_=x_d[128 * lt:128 * (lt + 1), :])
                xnb = xin.tile([128, HIDDEN], BF16, tag="x_natb")
                nc.vector.tensor_copy(xnb[:], xn[:])
                for c in range(HC):
                    tp = ps2.tile([128, 128], BF16, tag="s", name=f"tr_ps{lt}_{c}",
                                  padded_shape=[128, 1024])
                    nc.tensor.transpose(tp[:], xnb[:, 128 * c:128 * (c + 1)],
                                        ident_b[:])
                    nc.vector.tensor_copy(
                        x_t[c][:, 128 * lt:128 * (lt + 1)], tp[:]
                    )

            # ---------- phase 1.5: residual slice x^T (fp32) ----------
            xr_t_a = pers.tile([128, L], F32, tag="xr_t_a")
            xr_t_b = pers.tile([64, L], F32, tag="xr_t_b")
            for lt in range(LT):
                xrn = xin.tile([128, OSL], F32, tag="xr_nat")
                nc.gpsimd.dma_start(out=xrn[:],
                                    in_=xr_d[128 * lt:128 * (lt + 1), :])
                tp = ps_tile([128, 128], f"xr_ps{lt}_0")
                nc.tensor.transpose(tp[:], xrn[:, 0:128], ident[:])
                nc.vector.tensor_copy(xr_t_a[:, 128 * lt:128 * (lt + 1)], tp[:])
                tp2 = ps_tile([64, 128], f"xr_ps{lt}_1")
                nc.tensor.transpose(tp2[:], xrn[:, 128:192], ident[:])
                nc.vector.tensor_copy(xr_t_b[:, 128 * lt:128 * (lt + 1)], tp2[:])

            # ---------- phase 2: QKV projections ----------
            # q^T / k^T: [192, L] as a [128, L] + [64, L] pair
            q_a = pers.tile([128, L], BF16, tag="q_a")
            k_a = pers.tile([128, L], BF16, tag="k_a")
            q_b_t = pers.tile([64, L], BF16, tag="q_b")
            k_b_t = pers.tile([64, L], BF16, tag="k_b")
            q_b = q_b_t[:]
            k_b = k_b_t[:]
            for wi, (dst, w_sb, bcol) in enumerate((
                ((q_a[:], q_b), wq, 0),
                ((k_a[:], k_b), wk, 2),
            )):
                for fc in range(2):  # feature chunk: 0 -> 128 rows, 1 -> 64 rows
                    m = 128 if fc == 0 else 64
                    for half in range(2):
                        ps = ps_tile([m, 1024], f"qk_ps{wi}_{fc}_{half}")
                        for qt in range(2):
                            sl = slice(512 * qt, 512 * (qt + 1))
                            xsl = slice(1024 * half + 512 * qt,
                                        1024 * half + 512 * (qt + 1))
                            for c in range(HC):
                                nc.tensor.matmul(
                                    ps[:, sl],
                                    w_sb[:, c, 128 * fc:128 * fc + m],
                                    x_t[c][:, xsl],
                                    start=(c == 0),
                                    stop=(c == HC - 1),
                                )
                        nc.vector.tensor_scalar_add(
                            dst[fc][:, 1024 * half:1024 * (half + 1)], ps[:],
                            pcol[0:m, bcol + fc:bcol + fc + 1]
                        )

            # V' tiles: [128, 3*65] per l-tile; per head h cols 65h..65h+63 =
            # (x@wv + b)*mask, col 65h+64 = mask
            v_sb = [work.tile([128, 3 * 65], BF16, tag=f"v{lt}", name=f"v{lt}",
                              bufs=1)
                    for lt in range(LT)]
            for lt in range(LT):
                vp = ps_tile([128, HF], f"v_ps{lt}")
                for c in range(HC):
                    nc.tensor.matmul(
                        vp[:],
                        x_t[c][:, 128 * lt:128 * (lt + 1)],
                        wv[:, c, :],
                        start=(c == 0),
                        stop=False,
                    )
                # + wv_b broadcast over rows: ones_col^T (K=1) x bias row
                nc.tensor.matmul(
                    vp[:],
                    ones_bf[:, 0:128],
                    prow[:, 0:HF],
                    start=False,
                    stop=True,
                )
                for h in range(HPC):
                    nc.vector.tensor_scalar_mul(
                        v_sb[lt][:, 65 * h:65 * h + 64],
                        vp[:, 64 * h:64 * (h + 1)],
                        mask_cols[:, lt:lt + 1],
                    )
                    nc.vector.tensor_copy(
                        v_sb[lt][:, 65 * h + 64:65 * h + 65],
                        mask_cols[:, lt:lt + 1],
                    )

            # ---------- phase 3+4+5: attention / projection / split RS ----
            attn_a = pers.tile([128, L], BF16, tag="attn_a")  # heads 0,1
            attn_b = pers.tile([64, L], BF16, tag="attn_b")   # head 2

            def attn_normalize(av, h, qh, o_ap):
                q0 = 1024 * qh
                av_sb = work.tile([64, 1024], F32, tag="av_sb", bufs=2,
                                  name=f"avs{h}_{qh}")
                nc.scalar.copy(av_sb[:], av[0:64, :])
                l_sb = work.tile([1, 1024], F32, tag="l_sb", bufs=2,
                                 name=f"l{h}_{qh}")
                nc.scalar.copy(l_sb[:], av[64:65, :])
                r_row = work.tile([1, 1024], F32, tag="r_row", bufs=2,
                                  name=f"rr{h}_{qh}")
                nc.vector.reciprocal(r_row[:], l_sb[:])
                rb_sb = work.tile([64, 1024], F32, tag="rb_sb", bufs=2,
                                  name=f"rbs{h}_{qh}")
                nc.gpsimd.partition_broadcast(rb_sb[:], r_row[:])
                nc.vector.tensor_mul(
                    rb_sb[:], rb_sb[:], mask_bc[:, q0:q0 + 1024]
                )
                nc.vector.tensor_mul(
                    o_ap[:, q0:q0 + 1024], av_sb[:], rb_sb[:]
                )

            ln_state = {}

            def ln_chunk(qh):
                for pc, m in ((0, 128), (1, 64)):
                    xr_ap = xr_t_a[:] if pc == 0 else xr_t_b[:]
                    if qh == 0 and pc == 0:
                        ln_state['y0'] = work.tile([128, L], F32, tag="y0",
                                                   bufs=1, name="y0")
                        ln_state['y1'] = work.tile([64, L], F32, tag="y1",
                                                   bufs=1, name="y1")
                        ln_state['bn0'] = work.tile([128, 24], F32, tag="bn0",
                                                    bufs=1, name="bn0")
                        ln_state['bn1'] = work.tile([64, 24], F32, tag="bn1",
                                                    bufs=1, name="bn1")
                    y = ln_state[f'y{pc}']
                    bnst = ln_state[f'bn{pc}']
                    rs_ap = rs_qh[qh][:].rearrange("(r l) -> r l", l=1024)
                    yb = work.tile([m, 1024], F32, tag="yb", bufs=2,
                                   name=f"yb{pc}_{qh}")
                    nc.sync.dma_start(out=yb[:],
                                      in_=rs_ap[128 * pc:128 * pc + m, :])
                    nc.vector.tensor_add(
                        y[:, 1024 * qh:1024 * (qh + 1)],
                        xr_ap[:, 1024 * qh:1024 * (qh + 1)], yb[:]
                    )
                    for cch in range(2):
                        nc.vector.bn_stats(
                            bnst[:, 6 * (2 * qh + cch):6 * (2 * qh + cch + 1)],
                            y[:, 1024 * qh + 512 * cch:
                              1024 * qh + 512 * (cch + 1)],
                        )

            partial_qh = [
                nc.dram_tensor("partial_q0", [HIDDEN, 1024], F32),
                nc.dram_tensor("partial_q1", [HIDDEN, 1024], F32),
            ]
            rs_qh = [
                nc.dram_tensor("rs_out_q0", [OSL * 1024], F32),
                nc.dram_tensor("rs_out_q1", [OSL * 1024], F32),
            ]
            for qh in range(2):
                q0 = 1024 * qh
                # heads 0,1: row-group-packed scores (K=64 pairs), shared
                # exp tiles [h0 512q | h1 512q]
                av0 = pav.tile([65, 1024], F32, tag="av", bufs=2,
                               name=f"av0_{qh}")
                av1 = pav.tile([65, 1024], F32, tag="av", bufs=2,
                               name=f"av1_{qh}")
                for kt in range(LT):
                    ksl = slice(128 * kt, 128 * (kt + 1))
                    ptiles = []
                    for qq in range(2):
                        qsl = slice(q0 + 512 * qq, q0 + 512 * (qq + 1))
                        sp = ps_tile([128, 1024], f"s01_{qh}_{kt}_{qq}")
                        nc.tensor.matmul(sp[:, 0:512], k_a[0:64, ksl],
                                         q_a[0:64, qsl])
                        nc.tensor.matmul(sp[:, 512:1024], k_a[64:128, ksl],
                                         q_a[64:128, qsl])
                        pexp_t = pexp.tile([128, 1024], BF16, tag="p",
                                           name=f"p01_{qh}_{kt}_{qq}")
                        nc.scalar.activation(pexp_t[:], sp[:], AF.Exp,
                                             scale=0.125)
                        ptiles.append(pexp_t)
                    for hh, av in ((0, av0), (1, av1)):
                        for qq in range(2):
                            nc.tensor.matmul(
                                av[:, 512 * qq:512 * (qq + 1)],
                                v_sb[kt][:, 65 * hh:65 * (hh + 1)],
                                ptiles[qq][:, 512 * hh:512 * (hh + 1)],
                                start=(kt == 0),
                                stop=(kt == LT - 1),
                            )
                attn_normalize(av0, 0, qh, attn_a[0:64, :])
                attn_normalize(av1, 1, qh, attn_a[64:128, :])
                # head 2 (solo)
                av2 = pav.tile([65, 1024], F32, tag="av", bufs=2,
                               name=f"av2_{qh}")
                for kt in range(LT):
                    ksl = slice(128 * kt, 128 * (kt + 1))
                    sp = ps_tile([128, 1024], f"s2_{qh}_{kt}")
                    for qq in range(2):
                        qsl = slice(q0 + 512 * qq, q0 + 512 * (qq + 1))
                        nc.tensor.matmul(sp[:, 512 * qq:512 * (qq + 1)],
                                         k_b[:, ksl], q_b[:, qsl])
                    pexp_t = pexp.tile([128, 1024], BF16, tag="p",
                                       name=f"p2_{qh}_{kt}")
                    nc.scalar.activation(pexp_t[:], sp[:], AF.Exp, scale=0.125)
                    for qq in range(2):
                        nc.tensor.matmul(
                            av2[:, 512 * qq:512 * (qq + 1)],
                            v_sb[kt][:, 130:195],
                            pexp_t[:, 512 * qq:512 * (qq + 1)],
                            start=(kt == 0),
                            stop=(kt == LT - 1),
                        )
                attn_normalize(av2, 2, qh, attn_b[:])

                if qh == 1:
                    # half-0 layernorm chunk: RS0 finished during qh1's
                    # attention; emit here so the DVE queue stays clear
                    ln_chunk(0)

                # projection for this query half, then its ReduceScatter
                for oc in range(HC):
                    st = work.tile([128, 1024], F32, tag="stage", bufs=2,
                                   name=f"st{qh}_{oc}")
                    po = ps_tile([128, 1024], f"po{qh}_{oc}")
                    for qt in range(2):
                        sl = slice(512 * qt, 512 * (qt + 1))
                        asl = slice(q0 + 512 * qt, q0 + 512 * (qt + 1))
                        nc.tensor.matmul(
                            po[:, sl],
                            wo_a[:, 128 * oc:128 * (oc + 1)],
                            attn_a[:, asl],
                            start=True,
                            stop=False,
                        )
                        nc.tensor.matmul(
                            po[:, sl],
                            wo_b_sb[:, 128 * oc:128 * (oc + 1)],
                            attn_b[:, asl],
                            start=False,
                            stop=False,
                        )
                        # + wo_b/4 broadcast over columns
                        nc.tensor.matmul(
                            po[:, sl],
                            prow[:, HF + 128 * oc:HF + 128 * (oc + 1)],
                            ones_bf[:, 0:512],
                            start=False,
                            stop=True,
                        )
                    nc.vector.tensor_copy(st[:], po[:])
                    nc.gpsimd.dma_start(
                        out=partial_qh[qh][128 * oc:128 * (oc + 1), :],
                        in_=st[:],
                    )
                nc.gpsimd.collective_compute(
                    "ReduceScatter",
                    ALU.add,
                    replica_groups=[[0, 1, 2, 3], [4, 5, 6, 7]],
                    ins=[partial_qh[qh][:].opt()],
                    outs=[rs_qh[qh][:].opt()],
                )

            # ---------- layernorm over L (second half + finish) ----------
            ln_chunk(1)
            for pc, m in ((0, 128), (1, 64)):
                y = ln_state[f'y{pc}']
                bnst = ln_state[f'bn{pc}']
                stats = work.tile([m, 2], F32, tag=f"stats{pc}", bufs=1,
                                  name=f"stats{pc}")
                nc.vector.bn_aggr(stats[:], bnst[:])
                std = work.tile([m, 1], F32, tag=f"std{pc}", bufs=1,
                                name=f"std{pc}")
                nc.scalar.activation(
                    std[:], stats[:, 1:2], AF.Sqrt, scale=float(L) / float(L - 1)
                )
                rstd = work.tile([m, 1], F32, tag=f"rstd{pc}", bufs=1,
                                 name=f"rstd{pc}")
                nc.vector.reciprocal(rstd[:], std[:])
                ga = pcol[0:m, 12 + pc:13 + pc]
                be = pcol[0:m, 14 + pc:15 + pc]
                amul = work.tile([m, 1], F32, tag=f"amul{pc}", bufs=1,
                                 name=f"amul{pc}")
                nc.vector.tensor_mul(amul[:], rstd[:], ga)
                tmpb = work.tile([m, 1], F32, tag=f"tmpb{pc}", bufs=1,
                                 name=f"tmpb{pc}")
                nc.vector.tensor_mul(tmpb[:], stats[:, 0:1], amul[:])
                badd = work.tile([m, 1], F32, tag=f"badd{pc}", bufs=1,
                                 name=f"badd{pc}")
                nc.vector.tensor_sub(badd[:], be, tmpb[:])
                yo = work.tile([m, L], F32, tag="yo", bufs=1,
                               name=f"yo{pc}")
                nc.vector.tensor_scalar(
                    yo[:], y[:], amul[:], badd[:], op0=ALU.mult, op1=ALU.add
                )
                nc.sync.dma_start(out=out_d[128 * pc:128 * pc + m, :], in_=yo[:])

    nc.compile()
    return nc


_NC = None


def _get_nc():
    global _NC
    if _NC is None:
        _NC = build_nc()
    return _NC


def make_in_maps(inputs, attention_mask, wq_w, wq_b, wk_w, wk_b, wv_w, wv_b,
                 wo_w, wo_b, gamma, beta):
    x = np.asarray(inputs, np.float32)
    am = np.asarray(attention_mask, np.int32)
    in_maps = []
    for c in range(NCORES):
        b, g = c // 4, c % 4
        hsl = slice(HF * g, HF * (g + 1))
        pcol = np.zeros((128, 16), np.float32)
        for j, vec in ((0, np.asarray(wq_b)[hsl]), (2, np.asarray(wk_b)[hsl]),
                       (4, np.asarray(wv_b)[hsl])):
            pcol[:, j] = vec[:128]
            pcol[:64, j + 1] = vec[128:]
        wob4 = np.asarray(wo_b, np.float32) / 4.0
        pcol[:, 6:12] = wob4.reshape(6, 128).T
        for j, vec in ((12, np.asarray(gamma)[hsl]), (14, np.asarray(beta)[hsl])):
            pcol[:, j] = vec[:128]
            pcol[:64, j + 1] = vec[128:]
        prow = np.zeros((1, 960), BFNP)
        prow[0, :HF] = np.asarray(wv_b)[hsl]
        prow[0, HF:] = wob4
        in_maps.append({
            "x": np.ascontiguousarray(x[b]),
            "xr": np.ascontiguousarray(x[b][:, hsl]),
            "wq": np.ascontiguousarray(np.asarray(wq_w, np.float32)[:, hsl].astype(BFNP)),
            "wk": np.ascontiguousarray(np.asarray(wk_w, np.float32)[:, hsl].astype(BFNP)),
            "wv": np.ascontiguousarray(np.asarray(wv_w, np.float32)[:, hsl].astype(BFNP)),
            "wo_r": np.ascontiguousarray(np.asarray(wo_w, np.float32)[hsl, :].astype(BFNP)),
            "mask_i": np.ascontiguousarray(am[b][None, :]),
            "params_col": pcol,
            "params_row": prow,
        })
    return in_maps


def run(trace=False, **inputs):
    nc = _get_nc()
    in_maps = make_in_maps(**inputs)
    res = run_bass_kernel_spmd(nc, in_maps, core_ids=list(range(NCORES)),
                               trace=trace)
    out = np.zeros((B, L, HIDDEN), np.float32)
    for c in range(NCORES):
        b, g = c // 4, c % 4
        out[b, :, HF * g:HF * (g + 1)] = res.results[c]["out_t"].T
    return out, res


def kernel(**inputs):
    out, _ = run(trace=False, **inputs)
    return out



# revision 24
# speedup vs baseline: 1.9768x; 1.9768x over previous
"""Trainium2 Bass kernel for nn_MultiHeadAttention (B=2, L=2048, H=768, 12 heads).

Sharding (8 cores): core c -> batch b=c//4, heads 3*(c%4)..3*(c%4)+2.

v2: mask-compacted attention. The host permutes each batch's sequence so
unmasked positions come first (attention is permutation-equivariant and the
sequence-dim layernorm is permutation-invariant), so the device only runs
attention on the first N1 positions (N1 = padded count of unmasked slots).
Masked positions get the closed-form y = x + wo_b path with no collective.
Key masking is folded into the Exp activation bias (per-partition AP);
query masking is folded into the softmax reciprocal row. Partials are
reduce-scattered in bf16, chunked per query block so the collective overlaps
compute. Host passes x^T/xr^T directly (no on-device transposes).
"""

import os
import sys

import ml_dtypes
import numpy as np

BFNP = ml_dtypes.bfloat16

sys.path.insert(0, "/opt/trn_rl_repo")

import concourse.bass as bass  # noqa: E402
import concourse.bacc as bacc  # noqa: E402
import concourse.mybir as mybir  # noqa: E402
from concourse import tile  # noqa: E402
from concourse.bass_utils import run_bass_kernel_spmd  # noqa: E402

F32 = mybir.dt.float32
BF16 = mybir.dt.bfloat16
AF = mybir.ActivationFunctionType
ALU = mybir.AluOpType

HIDDEN = 768
HEADS = 12
HD = 64
L = 2048
B = 2
NCORES = 8
HPC = 3          # heads per core
HF = HPC * HD    # 192 features per core
HC = HIDDEN // 128  # 6 hidden chunks
OSL = HIDDEN // 4   # 192 output-slice rows per core
MASK_BIAS = -30.0


def _chunks(total, step):
    out = []
    o = 0
    while o < total:
        w = min(step, total - o)
        out.append((o, w))
        o += w
    return out


def build_nc(N1, has_vbias):
    _ks = os.environ.get("KSTAGE", "5")
    SUB = int(_ks[1:]) if len(_ks) > 1 else 9   # 21/22/23 sub-stages
    STAGE = int(_ks[0])
    NT = N1 // 128
    QBS = _chunks(N1, 512)

    nc = bacc.Bacc("TRN2", target_bir_lowering=False, debug=False,
                   num_devices=NCORES)

    xt_d = nc.dram_tensor("xt", [HIDDEN, N1], BF16, kind="ExternalInput")
    xr_d = nc.dram_tensor("xr", [OSL, L], F32, kind="ExternalInput")
    wqa_d = nc.dram_tensor("wqa", [HIDDEN, 128], BF16, kind="ExternalInput")
    wka_d = nc.dram_tensor("wka", [HIDDEN, 128], BF16, kind="ExternalInput")
    wqkb_d = nc.dram_tensor("wqkb", [HIDDEN, 128], BF16, kind="ExternalInput")
    wv_d = nc.dram_tensor("wv", [HIDDEN, HF], BF16, kind="ExternalInput")
    wo_d = nc.dram_tensor("wo_r", [HF, HIDDEN], BF16, kind="ExternalInput")
    # pcol[128,16]: 0=wq_b[:128] 1=wk_b[:128] 2=[wk_b[128:];wq_b[128:]]
    # 3=wo_b[hsl][:128] 4=wo_b[hsl][128:] (rows 0:64) 6..11=wo_b/4 (768)
    # 12,13=gamma 14,15=beta
    pcol_d = nc.dram_tensor("pcol", [128, 16], F32, kind="ExternalInput")
    mb_d = nc.dram_tensor("mb", [128, NT], F32, kind="ExternalInput")
    mrow_d = nc.dram_tensor("mrow", [1, N1], F32, kind="ExternalInput")
    if has_vbias:
        prow_d = nc.dram_tensor("prow", [1, HF], BF16, kind="ExternalInput")
    out_d = nc.dram_tensor("out_t", [OSL, L], F32, kind="ExternalOutput")

    partial_d = [nc.dram_tensor(f"partial_{i}", [HIDDEN, w], BF16)
                 for i, (_, w) in enumerate(QBS)]
    rs_d = [nc.dram_tensor(f"rs_{i}", [OSL * w], BF16)
            for i, (_, w) in enumerate(QBS)]

    with tile.TileContext(nc) as tc:
        with (
            tc.tile_pool(name="persist", bufs=1) as pers,
            tc.tile_pool(name="work", bufs=2) as work,
            tc.tile_pool(name="psc", bufs=2, space=bass.MemorySpace.PSUM) as psc,
            tc.tile_pool(name="pav", bufs=2, space=bass.MemorySpace.PSUM) as pav,
            tc.tile_pool(name="ppj", bufs=2, space=bass.MemorySpace.PSUM) as ppj,
            tc.tile_pool(name="pexp", bufs=3) as pexp,
        ):
            def s_tile(shape, name):
                return psc.tile(shape, F32, tag="s", name=name,
                                padded_shape=[128, 1024])

            # ---------- phase 0: params + weights ----------
            pcol = pers.tile([128, 16], F32, tag="pcol")
            nc.sync.dma_start(out=pcol[:], in_=pcol_d[:])
            mb = pers.tile([128, NT], F32, tag="mb")
            nc.sync.dma_start(out=mb[:], in_=mb_d[:])
            mrow = pers.tile([1, N1], F32, tag="mrow")
            nc.sync.dma_start(out=mrow[:], in_=mrow_d[:])
            if has_vbias:
                prow = pers.tile([1, HF], BF16, tag="prow")
                nc.sync.dma_start(out=prow[:], in_=prow_d[:])
                ones_bf = pers.tile([1, 128], BF16, tag="ones_bf")
                nc.vector.memset(ones_bf[:], 1.0)

            # prefetch exp table set during load phase
            warm = work.tile([1, 8], F32, tag="warm", bufs=1)
            nc.vector.memset(warm[:], 0.0)
            warm2 = work.tile([1, 8], F32, tag="warm2", bufs=1)
            nc.scalar.activation(warm2[:], warm[:], AF.Exp, scale=1.0)

            wqa = pers.tile([128, HC, 128], BF16, tag="wqa")
            wka = pers.tile([128, HC, 128], BF16, tag="wka")
            wqkb = pers.tile([128, HC, 128], BF16, tag="wqkb")
            wv = pers.tile([128, HC, HF], BF16, tag="wv")
            for w_sb, w_d in ((wqa, wqa_d), (wka, wka_d), (wqkb, wqkb_d),
                              (wv, wv_d)):
                nc.sync.dma_start(
                    out=w_sb[:], in_=w_d[:].rearrange("(c p) m -> p c m", p=128)
                )
            wo_a = pers.tile([128, HIDDEN], BF16, tag="wo_a")
            wo_b2 = pers.tile([64, HIDDEN], BF16, tag="wo_b2")
            nc.gpsimd.dma_start(out=wo_a[:], in_=wo_d[0:128, :])
            nc.gpsimd.dma_start(out=wo_b2[:], in_=wo_d[128:HF, :])

            # ---------- phase 1: residual slice + masked tail ----------
            xr_a = pers.tile([128, L], F32, tag="xr_a")
            xr_b = pers.tile([64, L], F32, tag="xr_b")
            nc.sync.dma_start(out=xr_a[:], in_=xr_d[0:128, :])
            nc.sync.dma_start(out=xr_b[:], in_=xr_d[128:OSL, :])

            y0 = pers.tile([128, L], F32, tag="y0")
            y1 = pers.tile([64, L], F32, tag="y1")
            # bn_aggr requires equal-size groups: four fixed 512-wide groups,
            # each emitted once its whole span has been written into y.
            NG = L // 512
            bn0 = pers.tile([128, 6 * NG], F32, tag="bn0")
            bn1 = pers.tile([64, 6 * NG], F32, tag="bn1")
            covered = []
            bn_done = set()

            def cover(lo, hi):
                covered.append((lo, hi))
                for g in range(NG):
                    glo, ghi = 512 * g, 512 * (g + 1)
                    if g in bn_done:
                        continue
                    need = set(range(glo, ghi, 128))
                    for (clo, chi) in covered:
                        need -= set(range(clo, chi, 128))
                    if not need:
                        bn_done.add(g)
                        if STAGE < 5:
                            continue
                        for pc, m, y, bnst in (
                            (0, 128, y0, bn0), (1, 64, y1, bn1),
                        ):
                            nc.vector.bn_stats(
                                bnst[0:m, 6 * g:6 * (g + 1)], y[0:m, glo:ghi]
                            )

            for (o, w) in _chunks(L - N1, 512):
                sl = slice(N1 + o, N1 + o + w)
                for pc, m, y, xr_ap, bcol in (
                    (0, 128, y0, xr_a, 3), (1, 64, y1, xr_b, 4),
                ):
                    nc.vector.tensor_scalar_add(
                        y[0:m, sl], xr_ap[0:m, sl], pcol[0:m, bcol:bcol + 1]
                    )
                cover(N1 + o, N1 + o + w)

            # ---------- phase 2: load x^T (column chunks) ----------
            x_sb = pers.tile([128, HC, N1], BF16, tag="x_sb")
            for (o, w) in QBS:
                nc.gpsimd.dma_start(
                    out=x_sb[:, :, o:o + w],
                    in_=xt_d[:, o:o + w].rearrange("(c p) m -> p c m", p=128),
                )

            # ---------- phase 3: QKV ----------
            q_a = pers.tile([128, N1], BF16, tag="q_a")
            k_a = pers.tile([128, N1], BF16, tag="k_a")
            q_b = pers.tile([64, N1], BF16, tag="q_b")
            k_b = pers.tile([64, N1], BF16, tag="k_b")
            for wi, (w_sb, dst, bcol) in enumerate(
                ((wqa, q_a, 0), (wka, k_a, 1), (wqkb, None, 2))
            ):
                for ci, (o, w) in enumerate(QBS):
                    ps = s_tile([128, w], f"qk{wi}_{ci}")
                    for c in range(HC):
                        nc.tensor.matmul(
                            ps[:], w_sb[:, c, :], x_sb[:, c, o:o + w],
                            start=(c == 0), stop=(c == HC - 1),
                        )
                    if dst is not None:
                        nc.vector.tensor_scalar_add(
                            dst[:, o:o + w], ps[:], pcol[:, bcol:bcol + 1]
                        )
                    else:
                        # psum rows 0:64 = k_h2, rows 64:128 = q_h2
                        nc.vector.tensor_scalar_add(
                            k_b[:, o:o + w], ps[0:64, :], pcol[0:64, 2:3]
                        )
                        nc.vector.tensor_scalar_add(
                            q_b[:, o:o + w], ps[64:128, :], pcol[64:128, 2:3]
                        )

            # V' tiles: [128, 3, 65]; per head h cols 0:64 = x@wv slice,
            # col 64 = 1.0 (denominator accumulator row)
            v_sb = pers.tile([128, NT, HPC, 65], BF16, tag="v_sb")
            nc.vector.memset(v_sb[:, :, :, 64:65], 1.0)
            for kt in range(NT):
                ksl = slice(128 * kt, 128 * (kt + 1))
                vp = s_tile([128, HF], f"v_{kt}")
                for c in range(HC):
                    nc.tensor.matmul(
                        vp[:], x_sb[:, c, ksl], wv[:, c, :],
                        start=(c == 0),
                        stop=(c == HC - 1 and not has_vbias),
                    )
                if has_vbias:
                    nc.tensor.matmul(vp[:], ones_bf[:], prow[:],
                                     start=False, stop=True)
                nc.vector.tensor_copy(
                    v_sb[:, kt, :, 0:64],
                    vp[:].rearrange("p (h f) -> p h f", h=HPC),
                )

            # ---------- phase 4: attention + projection + RS ----------
            attn_a = pers.tile([128, N1], BF16, tag="attn_a")
            attn_b = pers.tile([64, N1], BF16, tag="attn_b")

            def normalize(av, qo, qw, o_ap, tag):
                # attn = av[0:64] * (1/l) * qmask, l = av[64]
                rr = work.tile([1, qw], F32, tag="rr", bufs=2, name=f"rr{tag}")
                nc.vector.reciprocal(rr[:], av[64:65, :])
                rq = work.tile([1, qw], F32, tag="rq", bufs=2, name=f"rq{tag}")
                nc.vector.tensor_mul(rq[:], rr[:], mrow[:, qo:qo + qw])
                rb = work.tile([64, qw], F32, tag="rb", bufs=2, name=f"rb{tag}")
                nc.gpsimd.partition_broadcast(rb[:], rq[:])
                nc.vector.tensor_mul(o_ap, av[0:64, :], rb[:])

            def ln_chunk(i):
                o, w = QBS[i]
                rs_ap = rs_d[i][:].rearrange("(r l) -> r l", l=w)
                for pc, m, y, xr_ap in (
                    (0, 128, y0, xr_a), (1, 64, y1, xr_b),
                ):
                    yb = work.tile([m, w], BF16, tag="yb", bufs=2,
                                   name=f"yb{i}_{pc}")
                    nc.sync.dma_start(out=yb[:],
                                      in_=rs_ap[128 * pc:128 * pc + m, :])
                    nc.vector.tensor_add(
                        y[0:m, o:o + w], xr_ap[0:m, o:o + w], yb[:]
                    )
                cover(o, o + w)

            for qi, (qo, qw) in enumerate(QBS):
                if STAGE < 2:
                    break
                qsl = slice(qo, qo + qw)
                # ---- heads 0,1 ----
                av0 = pav.tile([65, qw], F32, tag="av", name=f"av0_{qi}",
                               padded_shape=[65, 512])
                av1 = pav.tile([65, qw], F32, tag="av", name=f"av1_{qi}",
                               padded_shape=[65, 512])
                exps = []
                for kt in range(NT):
                    ksl = slice(128 * kt, 128 * (kt + 1))
                    # h1 scores always start at col 512 (a fresh PSUM bank):
                    # the two row-tiled matmuls run concurrently on the PE,
                    # and concurrent writes into one bank are a HW error.
                    sp = s_tile([128, 512 + qw], f"s01_{qi}_{kt}")
                    nc.tensor.matmul(sp[:, 0:qw], k_a[0:64, ksl],
                                     q_a[0:64, qsl])
                    nc.tensor.matmul(sp[:, 512:512 + qw], k_a[64:128, ksl],
                                     q_a[64:128, qsl])
                    pe = pexp.tile([128, 512 + qw], BF16, tag="p",
                                   name=f"p01_{qi}_{kt}")
                    if qw == 512:
                        nc.scalar.activation(pe[:], sp[:], AF.Exp, scale=0.125,
                                             bias=mb[:, kt:kt + 1])
                    else:
                        for ho in (0, 512):
                            nc.scalar.activation(
                                pe[:, ho:ho + qw], sp[:, ho:ho + qw],
                                AF.Exp, scale=0.125, bias=mb[:, kt:kt + 1])
                    exps.append(pe)
                    # AV for previous kt (software pipeline: keep PE ahead)
                    if kt > 0:
                        pprev = exps[kt - 1]
                        for hh, av in ((0, av0), (1, av1)):
                            nc.tensor.matmul(
                                av[:], v_sb[:, kt - 1, hh, :],
                                pprev[:, 512 * hh:512 * hh + qw],
                                start=(kt - 1 == 0), stop=False,
                            )
                for hh, av in ((0, av0), (1, av1)):
                    nc.tensor.matmul(
                        av[:], v_sb[:, NT - 1, hh, :],
                        exps[NT - 1][:, 512 * hh:512 * hh + qw],
                        start=(NT == 1), stop=True,
                    )
                normalize(av0, qo, qw, attn_a[0:64, qsl], f"0_{qi}")
                normalize(av1, qo, qw, attn_a[64:128, qsl], f"1_{qi}")

                # ---- head 2 ----
                av2 = pav.tile([65, qw], F32, tag="av", name=f"av2_{qi}",
                               padded_shape=[65, 512])
                exps2 = []
                for kt in range(NT):
                    ksl = slice(128 * kt, 128 * (kt + 1))
                    sp = s_tile([128, qw], f"s2_{qi}_{kt}")
                    nc.tensor.matmul(sp[:], k_b[:, ksl], q_b[:, qsl])
                    pe = pexp.tile([128, qw], BF16, tag="p",
                                   name=f"p2_{qi}_{kt}")
                    nc.scalar.activation(pe[:], sp[:], AF.Exp, scale=0.125,
                                         bias=mb[:, kt:kt + 1])
                    exps2.append(pe)
                    if kt > 0:
                        nc.tensor.matmul(
                            av2[:], v_sb[:, kt - 1, 2, :], exps2[kt - 1][:],
                            start=(kt - 1 == 0), stop=False,
                        )
                nc.tensor.matmul(
                    av2[:], v_sb[:, NT - 1, 2, :], exps2[NT - 1][:],
                    start=(NT == 1), stop=True,
                )
                normalize(av2, qo, qw, attn_b[:, qsl], f"2_{qi}")

                # ---- projection + ReduceScatter for this block ----
                if STAGE < 3:
                    continue
                for oc in range(HC):
                    osl = slice(128 * oc, 128 * (oc + 1))
                    po = ppj.tile([128, qw], F32, tag="pp", name=f"po{qi}_{oc}",
                                  padded_shape=[128, 512])
                    nc.tensor.matmul(po[:], wo_a[:, osl], attn_a[:, qsl],
                                     start=True, stop=False)
                    nc.tensor.matmul(po[:], wo_b2[:, osl], attn_b[:, qsl],
                                     start=False, stop=True)
                    st = work.tile([128, qw], BF16, tag="st", bufs=3,
                                   name=f"st{qi}_{oc}")
                    nc.vector.tensor_scalar_add(
                        st[:], po[:], pcol[:, 6 + oc:7 + oc]
                    )
                    nc.gpsimd.dma_start(out=partial_d[qi][osl, :], in_=st[:])
                if STAGE >= 4:
                    nc.gpsimd.collective_compute(
                        "ReduceScatter",
                        ALU.add,
                        replica_groups=[[0, 1, 2, 3], [4, 5, 6, 7]],
                        ins=[partial_d[qi][:].opt()],
                        outs=[rs_d[qi][:].opt()],
                    )
                # layernorm chunk for the previous block (its RS is done)
                if STAGE >= 5 and qi > 0:
                    ln_chunk(qi - 1)

            # ---------- phase 5: finish layernorm ----------
            if STAGE < 5:
                for pc, m, y in ((0, 128, y0), (1, 64, y1)):
                    nc.vector.memset(y[0:m, 0:N1], 0.5)
                    nc.sync.dma_start(out=out_d[128 * pc:128 * pc + m, :],
                                      in_=y[0:m, :])
            if STAGE >= 5:
                ln_chunk(len(QBS) - 1)
            for pc, m, y, bnst in (((0, 128, y0, bn0), (1, 64, y1, bn1))
                                   if STAGE >= 5 else ()):
                stats = work.tile([m, 2], F32, tag=f"stats{pc}", bufs=1)
                nc.vector.bn_aggr(stats[:], bnst[0:m, :])
                std = work.tile([m, 1], F32, tag=f"std{pc}", bufs=1)
                nc.scalar.activation(
                    std[:], stats[:, 1:2], AF.Sqrt,
                    scale=float(L) / float(L - 1)
                )
                rstd = work.tile([m, 1], F32, tag=f"rstd{pc}", bufs=1)
                nc.vector.reciprocal(rstd[:], std[:])
                amul = work.tile([m, 1], F32, tag=f"amul{pc}", bufs=1)
                nc.vector.tensor_mul(amul[:], rstd[:], pcol[0:m, 12 + pc:13 + pc])
                tmpb = work.tile([m, 1], F32, tag=f"tmpb{pc}", bufs=1)
                nc.vector.tensor_mul(tmpb[:], stats[:, 0:1], amul[:])
                badd = work.tile([m, 1], F32, tag=f"badd{pc}", bufs=1)
                nc.vector.tensor_sub(badd[:], pcol[0:m, 14 + pc:15 + pc], tmpb[:])
                for (o, w) in _chunks(L, 1024):
                    yo = work.tile([m, w], F32, tag="yo", bufs=2,
                                   name=f"yo{pc}_{o}")
                    nc.vector.tensor_scalar(
                        yo[:], y[0:m, o:o + w], amul[:], badd[:],
                        op0=ALU.mult, op1=ALU.add
                    )
                    nc.sync.dma_start(
                        out=out_d[128 * pc:128 * pc + m, o:o + w], in_=yo[:]
                    )

    nc.compile()
    return nc


_NC_CACHE = {}


def _get_nc(N1, has_vbias):
    key = (N1, has_vbias, os.environ.get("KSTAGE", "5"))
    if key not in _NC_CACHE:
        _NC_CACHE[key] = build_nc(N1, has_vbias)
    return _NC_CACHE[key]


def _prep(inputs, attention_mask, wq_w, wq_b, wk_w, wk_b, wv_w, wv_b,
          wo_w, wo_b, gamma, beta):
    x = np.asarray(inputs, np.float32)
    am = np.asarray(attention_mask, np.int32)
    wq_w = np.asarray(wq_w, np.float32)
    wk_w = np.asarray(wk_w, np.float32)
    wv_w = np.asarray(wv_w, np.float32)
    wo_w = np.asarray(wo_w, np.float32)
    wq_b = np.asarray(wq_b, np.float32)
    wk_b = np.asarray(wk_b, np.float32)
    wv_b = np.asarray(wv_b, np.float32)
    wo_b = np.asarray(wo_b, np.float32)
    gamma = np.asarray(gamma, np.float32)
    beta = np.asarray(beta, np.float32)

    perms, n1s = [], []
    for b in range(B):
        m = am[b]
        perm = np.argsort(-m, kind="stable").astype(np.int64)
        perms.append(perm)
        n1s.append(int(m.sum()))
    N1 = max(128, -(-max(max(n1s), 1) // 128) * 128)
    NT = N1 // 128
    has_vbias = bool(np.any(wv_b))

    in_maps = []
    for c in range(NCORES):
        b, g = c // 4, c % 4
        hsl = slice(HF * g, HF * (g + 1))
        perm = perms[b]
        xp = x[b][perm]                      # [L, 768] permuted
        mp = am[b][perm].astype(np.float32)  # permuted mask

        pcol = np.zeros((128, 16), np.float32)
        pcol[:, 0] = wq_b[hsl][:128]
        pcol[:, 1] = wk_b[hsl][:128]
        pcol[:64, 2] = wk_b[hsl][128:]
        pcol[64:, 2] = wq_b[hsl][128:]
        pcol[:, 3] = wo_b[hsl][:128]
        pcol[:64, 4] = wo_b[hsl][128:]
        pcol[:, 6:12] = (wo_b / 4.0).reshape(6, 128).T
        pcol[:, 12] = gamma[hsl][:128]
        pcol[:64, 13] = gamma[hsl][128:]
        pcol[:, 14] = beta[hsl][:128]
        pcol[:64, 15] = beta[hsl][128:]

        # 0 for unmasked keys, MASK_BIAS for masked ones
        mb = np.ascontiguousarray(
            (1.0 - mp[:N1].reshape(NT, 128).T) * MASK_BIAS, np.float32)
        mrow = np.ascontiguousarray(mp[None, :N1], np.float32)

        wq_s = wq_w[:, hsl]
        wk_s = wk_w[:, hsl]
        wqkb = np.concatenate([wk_s[:, 128:], wq_s[:, 128:]], axis=1)

        im = {
            "xt": np.ascontiguousarray(xp[:N1].T.astype(BFNP)),
            "xr": np.ascontiguousarray(xp[:, hsl].T),
            "wqa": np.ascontiguousarray(wq_s[:, :128].astype(BFNP)),
            "wka": np.ascontiguousarray(wk_s[:, :128].astype(BFNP)),
            "wqkb": np.ascontiguousarray(wqkb.astype(BFNP)),
            "wv": np.ascontiguousarray(wv_w[:, hsl].astype(BFNP)),
            "wo_r": np.ascontiguousarray(wo_w[hsl, :].astype(BFNP)),
            "pcol": pcol,
            "mb": mb,
            "mrow": mrow,
        }
        if has_vbias:
            im["prow"] = np.ascontiguousarray(wv_b[None, hsl].astype(BFNP))
        in_maps.append(im)
    return N1, has_vbias, perms, in_maps


def run(trace=False, **inputs):
    N1, has_vbias, perms, in_maps = _prep(**inputs)
    nc = _get_nc(N1, has_vbias)
    res = run_bass_kernel_spmd(nc, in_maps, core_ids=list(range(NCORES)),
                               trace=trace)
    out = np.zeros((B, L, HIDDEN), np.float32)
    for c in range(NCORES):
        b, g = c // 4, c % 4
        out[b][perms[b], HF * g:HF * (g + 1)] = res.results[c]["out_t"].T
    return out, res


def kernel(**inputs):
    out, _ = run(trace=False, **inputs)
    return out
